# revision 1
# baseline (speedup 1.0000x reference)
"""SwiGLU FFN (gate/up/down) on 8 TRN2 NeuronCores.

Strategy: data-parallel over tokens. Each core gets 1024 tokens and the
full weight set. All matmuls run in bf16 with fp32 PSUM accumulation.

Layout trick: activations are kept transposed on-chip (feature dim on
partitions, tokens on the free dim), so every matmul has its contraction
dim on partitions for both operands and no on-device transposes are
needed:
  gate.T = Wg_lhsT.T @ x.T    (lhsT[k,m] = Wg[m,k], k = hidden)
  h.T    = silu(gate.T) * up.T
  y.T    = Wd_lhsT.T @ h.T    (lhsT[k,m] = Wd[m,k], k = inter)

Weights are pre-tiled on the host into [m_tile, p, (g), k_tile, m] order
so each per-m-tile DMA reads 16-22KB contiguous per partition.

SBUF budget per partition (of ~208KB usable): xT 32KB + hT 86KB +
weight slots 2x22KB + staging ~12KB.
"""

import numpy as np
import ml_dtypes

import concourse.bass as bass
import concourse.tile as tile
import concourse.mybir as mybir
from concourse.bass_utils import run_bass_kernel_spmd

BF16 = ml_dtypes.bfloat16

P = 128
HID = 4096
INT = 11008
TOK = 8192
NCORES = 8
TPC = TOK // NCORES          # tokens per core
T = 512                      # tokens per pass (PSUM free-dim limit, f32)
NPASS = TPC // T
KTH = HID // P               # 32 k-tiles over hidden
MTI = INT // P               # 86 m-tiles over intermediate
MTH = HID // P               # 32 m-tiles over hidden (down proj)
KTI = INT // P               # 86 k-tiles over intermediate


def _split_multiwaits(nc):
    # This walrus build supports a single sync-wait slot per instruction;
    # hoist extra waits onto single-wait NoOps inserted just before the
    # offending instruction on the same engine (same semantics: the engine
    # stream blocks on each wait in order).
    n = 0
    for f in nc.m.functions:
        for blk in f.blocks:
            insts = blk.instructions  # live list
            i = 0
            while i < len(insts):
                inst = insts[i]
                si = getattr(inst, "sync_info", None)
                if si is not None and si.on_wait and len(si.on_wait) > 1:
                    waits = list(si.on_wait)
                    for j, w in enumerate(waits[:-1]):
                        nop = mybir.InstNoOp(
                            name=f"{inst.name}_splitwait{j}", ins=[], outs=[]
                        )
                        nop.engine = inst.engine
                        nop.sync_info = mybir.SyncInfo(on_wait=[w], on_update=[])
                        insts.insert(i, nop)
                        i += 1
                        n += 1
                    inst.sync_info = mybir.SyncInfo(
                        on_wait=[waits[-1]], on_update=list(si.on_update)
                    )
                i += 1
    return n


def build_nc():
    bf = mybir.dt.bfloat16
    f32 = mybir.dt.float32
    nc = bass.Bass()

    xt = nc.dram_tensor("xt", [NPASS, P, KTH, T], bf, kind="ExternalInput")
    wgu = nc.dram_tensor("wgu", [MTI, P, 2, KTH, P], bf, kind="ExternalInput")
    wd = nc.dram_tensor("wd", [MTH, P, KTI, P], bf, kind="ExternalInput")
    yt = nc.dram_tensor("yt", [NPASS, MTH, P, T], f32, kind="ExternalOutput")

    with tile.TileContext(nc) as tc:
        with (
            tc.tile_pool(name="xp", bufs=1) as xp,
            tc.tile_pool(name="hp", bufs=1) as hp,
            tc.tile_pool(name="wp", bufs=2) as wp,
            tc.tile_pool(name="sp", bufs=3) as sp,
            tc.tile_pool(name="pg", bufs=2, space="PSUM") as pg,
            tc.tile_pool(name="py", bufs=2, space="PSUM") as py,
        ):
            for ps in range(NPASS):
                xt_sb = xp.tile([P, KTH, T], bf, name="xt_sb", tag="xt_sb")
                nc.sync.dma_start(xt_sb[:], xt[ps])
                ht = hp.tile([P, MTI, T], bf, name="ht", tag="ht")
                for mt in range(MTI):
                    w = wp.tile([P, 2, KTH, P], bf, name="w_gu", tag="w")
                    nc.sync.dma_start(w[:], wgu[mt])
                    g_ps = pg.tile([P, T], f32, name="g_ps", tag="g")
                    u_ps = pg.tile([P, T], f32, name="u_ps", tag="u")
                    for kt in range(KTH):
                        nc.tensor.matmul(
                            g_ps[:], w[:, 0, kt], xt_sb[:, kt],
                            start=(kt == 0), stop=(kt == KTH - 1),
                        )
                    for kt in range(KTH):
                        nc.tensor.matmul(
                            u_ps[:], w[:, 1, kt], xt_sb[:, kt],
                            start=(kt == 0), stop=(kt == KTH - 1),
                        )
                    sil = sp.tile([P, T], f32, name="sil", tag="sil")
                    nc.scalar.activation(
                        sil[:], g_ps[:], mybir.ActivationFunctionType.Silu
                    )
                    nc.vector.tensor_mul(ht[:, mt], sil[:], u_ps[:])
                for mh in range(MTH):
                    wdt = wp.tile([P, KTI, P], bf, name="w_d", tag="w")
                    nc.sync.dma_start(wdt[:], wd[mh])
                    y_ps = py.tile([P, T], f32, name="y_ps", tag="y")
                    for kt in range(KTI):
                        nc.tensor.matmul(
                            y_ps[:], wdt[:, kt], ht[:, kt],
                            start=(kt == 0), stop=(kt == KTI - 1),
                        )
                    y_sb = sp.tile([P, T], f32, name="y_sb", tag="ysb")
                    nc.vector.tensor_copy(y_sb[:], y_ps[:])
                    nc.sync.dma_start(yt[ps, mh], y_sb[:])

    _split_multiwaits(nc)
    return nc


def prep_inputs(x, W_gate, W_up, W_down):
    # lhsT layouts: element [mt, p, (g,) kt, m] = W[mt*128+m, kt*128+p]
    wg = W_gate.reshape(MTI, P, KTH, P).transpose(0, 3, 2, 1)
    wu = W_up.reshape(MTI, P, KTH, P).transpose(0, 3, 2, 1)
    wgu = np.stack([wg, wu], axis=2).astype(BF16)          # [mt, p, 2, kt, m]
    wd = W_down.reshape(MTH, P, KTI, P).transpose(0, 3, 2, 1).astype(BF16)
    # x: [core, pass, t, kt, p] -> per-core [pass, p, kt, t]
    xr = x.reshape(NCORES, NPASS, T, KTH, P)
    xts = [np.ascontiguousarray(xr[c].transpose(0, 3, 2, 1)).astype(BF16)
           for c in range(NCORES)]
    return xts, wgu, wd


_NC_CACHE = []


def get_nc():
    if not _NC_CACHE:
        _NC_CACHE.append(build_nc())
    return _NC_CACHE[0]


def kernel(x, W_gate, W_up, W_down):
    x = np.asarray(x, dtype=np.float32)
    xts, wgu, wd = prep_inputs(
        np.asarray(x, np.float32),
        np.asarray(W_gate, np.float32),
        np.asarray(W_up, np.float32),
        np.asarray(W_down, np.float32),
    )
    nc = get_nc()
    in_maps = [{"xt": xts[c], "wgu": wgu, "wd": wd} for c in range(NCORES)]
    res = run_bass_kernel_spmd(nc, in_maps, core_ids=list(range(NCORES)))
    out = np.empty((TOK, HID), np.float32)
    for c in range(NCORES):
        ytc = res.results[c]["yt"]                          # [pass, mh, p, t]
        out[c * TPC:(c + 1) * TPC] = (
            ytc.transpose(0, 3, 1, 2).reshape(TPC, HID)
        )
    return out



# revision 5
# speedup vs baseline: 1.0104x; 1.0104x over previous
"""SwiGLU FFN (gate/up/down) on 8 TRN2 NeuronCores.

Strategy: data-parallel over tokens. Each core gets 1024 tokens and the
full weight set. All matmuls run in bf16 with fp32 PSUM accumulation.

Layout trick: activations are kept transposed on-chip (feature dim on
partitions, tokens on the free dim), so every matmul has its contraction
dim on partitions for both operands and no on-device transposes are
needed:
  gate.T = Wg_lhsT.T @ x.T    (lhsT[k,m] = Wg[m,k], k = hidden)
  h.T    = silu(gate.T) * up.T
  y.T    = Wd_lhsT.T @ h.T    (lhsT[k,m] = Wd[m,k], k = inter)

Weights are pre-tiled on the host into [m_tile, p, (g), k_tile, m] order
so each per-m-tile DMA reads 16-22KB contiguous per partition.

SBUF budget per partition (of ~208KB usable): xT 32KB + hT 86KB +
weight slots 2x22KB + staging ~12KB.
"""

import numpy as np
import ml_dtypes

import concourse.bass as bass
import concourse.tile as tile
import concourse.mybir as mybir
from concourse.bass_utils import run_bass_kernel_spmd

BF16 = ml_dtypes.bfloat16

P = 128
HID = 4096
INT = 11008
TOK = 8192
NCORES = 8
TPC = TOK // NCORES          # tokens per core
T = 512                      # tokens per pass (PSUM free-dim limit, f32)
NPASS = TPC // T
KTH = HID // P               # 32 k-tiles over hidden
MTI = INT // P               # 86 m-tiles over intermediate
MTH = HID // P               # 32 m-tiles over hidden (down proj)
KTI = INT // P               # 86 k-tiles over intermediate


def _split_multiwaits(nc):
    # This walrus build supports a single sync-wait slot per instruction;
    # hoist extra waits onto single-wait NoOps inserted just before the
    # offending instruction on the same engine (same semantics: the engine
    # stream blocks on each wait in order).
    n = 0
    for f in nc.m.functions:
        for blk in f.blocks:
            insts = blk.instructions  # live list
            i = 0
            while i < len(insts):
                inst = insts[i]
                si = getattr(inst, "sync_info", None)
                if si is not None and si.on_wait and len(si.on_wait) > 1:
                    waits = list(si.on_wait)
                    for j, w in enumerate(waits[:-1]):
                        nop = mybir.InstNoOp(
                            name=f"{inst.name}_splitwait{j}", ins=[], outs=[]
                        )
                        nop.engine = inst.engine
                        nop.sync_info = mybir.SyncInfo(on_wait=[w], on_update=[])
                        insts.insert(i, nop)
                        i += 1
                        n += 1
                    inst.sync_info = mybir.SyncInfo(
                        on_wait=[waits[-1]], on_update=list(si.on_update)
                    )
                i += 1
    return n


def build_nc():
    bf = mybir.dt.bfloat16
    f32 = mybir.dt.float32
    nc = bass.Bass()

    xt = nc.dram_tensor("xt", [NPASS, P, KTH, T], bf, kind="ExternalInput")
    wgu = nc.dram_tensor("wgu", [MTI, P, 2, KTH, P], bf, kind="ExternalInput")
    wd = nc.dram_tensor("wd", [MTH, P, KTI, P], bf, kind="ExternalInput")
    yt = nc.dram_tensor("yt", [NPASS, MTH, P, T], f32, kind="ExternalOutput")

    XCH = 4                      # xt DMA chunks (8 k-tiles each)
    with tile.TileContext(nc) as tc:
        with (
            tc.tile_pool(name="xp", bufs=1) as xp,
            tc.tile_pool(name="hp", bufs=1) as hp,
            tc.tile_pool(name="wp", bufs=2) as wp,
            tc.tile_pool(name="sp", bufs=3) as sp,
            tc.tile_pool(name="pg", bufs=2, space="PSUM") as pg,
            tc.tile_pool(name="py", bufs=2, space="PSUM") as py,
        ):
            for ps in range(NPASS):
                # mt=0 weights first: the first gate matmul needs w0-gate +
                # xt chunk 0, so get w0 moving before the bulk of xt.
                w0 = wp.tile([P, 2, KTH, P], bf, name="w_gu", tag="w")
                nc.sync.dma_start(w0[:, 0], wgu[0, :, 0])
                xt_sb = xp.tile([P, KTH, T], bf, name="xt_sb", tag="xt_sb")
                kch = KTH // XCH
                for c in range(XCH):
                    nc.sync.dma_start(
                        xt_sb[:, c * kch:(c + 1) * kch],
                        xt[ps, :, c * kch:(c + 1) * kch],
                    )
                nc.sync.dma_start(w0[:, 1], wgu[0, :, 1])
                ht = hp.tile([P, MTI, T], bf, name="ht", tag="ht")
                for mt in range(MTI):
                    if mt == 0:
                        w = w0
                    else:
                        w = wp.tile([P, 2, KTH, P], bf, name="w_gu", tag="w")
                        nc.sync.dma_start(w[:, 0], wgu[mt, :, 0])
                        nc.sync.dma_start(w[:, 1], wgu[mt, :, 1])
                    g_ps = pg.tile([P, T], f32, name="g_ps", tag="g")
                    u_ps = pg.tile([P, T], f32, name="u_ps", tag="u")
                    for kt in range(KTH):
                        nc.tensor.matmul(
                            g_ps[:], w[:, 0, kt], xt_sb[:, kt],
                            start=(kt == 0), stop=(kt == KTH - 1),
                        )
                    for kt in range(KTH):
                        nc.tensor.matmul(
                            u_ps[:], w[:, 1, kt], xt_sb[:, kt],
                            start=(kt == 0), stop=(kt == KTH - 1),
                        )
                    sil = sp.tile([P, T], f32, name="sil", tag="sil")
                    nc.scalar.activation(
                        sil[:], g_ps[:], mybir.ActivationFunctionType.Silu
                    )
                    nc.vector.tensor_mul(ht[:, mt], sil[:], u_ps[:])
                for mh in range(MTH):
                    wdt = wp.tile([P, KTI, P], bf, name="w_d", tag="w")
                    nc.sync.dma_start(wdt[:], wd[mh])
                    last = ps == NPASS - 1 and mh == MTH - 1
                    if not last:
                        y_ps = py.tile([P, T], f32, name="y_ps", tag="y")
                        for kt in range(KTI):
                            nc.tensor.matmul(
                                y_ps[:], wdt[:, kt], ht[:, kt],
                                start=(kt == 0), stop=(kt == KTI - 1),
                            )
                        y_sb = sp.tile([P, T], f32, name="y_sb", tag="ysb")
                        nc.vector.tensor_copy(y_sb[:], y_ps[:])
                        nc.sync.dma_start(yt[ps, mh], y_sb[:])
                    else:
                        # final tile: two half-N groups so the copy+DMA of
                        # the first half hides under the second half's MMs
                        T2 = T // 2
                        for hh in range(2):
                            y_ps = py.tile([P, T2], f32, name="y_ps2", tag="y2")
                            for kt in range(KTI):
                                nc.tensor.matmul(
                                    y_ps[:], wdt[:, kt],
                                    ht[:, kt, hh * T2:(hh + 1) * T2],
                                    start=(kt == 0), stop=(kt == KTI - 1),
                                )
                            y_sb = sp.tile([P, T2], f32, name="y_sb2", tag="ysb")
                            nc.vector.tensor_copy(y_sb[:], y_ps[:])
                            nc.sync.dma_start(
                                yt[ps, mh, :, hh * T2:(hh + 1) * T2], y_sb[:]
                            )

    _split_multiwaits(nc)
    return nc


def prep_inputs(x, W_gate, W_up, W_down):
    # lhsT layouts: element [mt, p, (g,) kt, m] = W[mt*128+m, kt*128+p]
    wg = W_gate.reshape(MTI, P, KTH, P).transpose(0, 3, 2, 1)
    wu = W_up.reshape(MTI, P, KTH, P).transpose(0, 3, 2, 1)
    wgu = np.stack([wg, wu], axis=2).astype(BF16)          # [mt, p, 2, kt, m]
    wd = W_down.reshape(MTH, P, KTI, P).transpose(0, 3, 2, 1).astype(BF16)
    # x: [core, pass, t, kt, p] -> per-core [pass, p, kt, t]
    xr = x.reshape(NCORES, NPASS, T, KTH, P)
    xts = [np.ascontiguousarray(xr[c].transpose(0, 3, 2, 1)).astype(BF16)
           for c in range(NCORES)]
    return xts, wgu, wd


_NC_CACHE = []


def get_nc():
    if not _NC_CACHE:
        _NC_CACHE.append(build_nc())
    return _NC_CACHE[0]


def kernel(x, W_gate, W_up, W_down):
    x = np.asarray(x, dtype=np.float32)
    xts, wgu, wd = prep_inputs(
        np.asarray(x, np.float32),
        np.asarray(W_gate, np.float32),
        np.asarray(W_up, np.float32),
        np.asarray(W_down, np.float32),
    )
    nc = get_nc()
    in_maps = [{"xt": xts[c], "wgu": wgu, "wd": wd} for c in range(NCORES)]
    res = run_bass_kernel_spmd(nc, in_maps, core_ids=list(range(NCORES)))
    out = np.empty((TOK, HID), np.float32)
    for c in range(NCORES):
        ytc = res.results[c]["yt"]                          # [pass, mh, p, t]
        out[c * TPC:(c + 1) * TPC] = (
            ytc.transpose(0, 3, 1, 2).reshape(TPC, HID)
        )
    return out

